# revision 17
# baseline (speedup 1.0000x reference)
"""GCN message-passing kernel for 8 TRN2 NeuronCores.

Strategy (graph/data parallel, per sharding hint):
  - Nodes are dst-sharded across 8 cores (12,500 each, padded to 12,544).
  - Per GCN layer, each core computes m = (h @ W) * deg_inv_sqrt for its
    local nodes (TensorE, feature-major), transposes to node-major, and
    AllGathers m into a full [100352, 128] node-major table.
  - Edge aggregation: dma_gather of 512B source rows (int16 indices, 4
    chunks of 25,088 rows each) + dma_scatter_add into the local agg
    table.  agg is initialized with the core's own m rows, which realizes
    the GCN self-loop term.
  - Epilogue: h_next = relu(deg_inv_sqrt * agg + b), transposed back to
    feature-major for the next layer's matmul.
  - Edges are grouped host-side so that all edges sharing a dst node live
    on the same of the 16 index lanes (one lane per SDMA engine) - this
    keeps read-modify-write scatter updates to any given row ordered on a
    single engine.

Host-side work is limited to sharding/layout: edge partitioning, index
lane packing, degree counts (CSR metadata), transposes of final outputs.
All FLOPs (matmuls, normalization, aggregation, bias, relu) run on
device.
"""

import os
import sys

import numpy as np

sys.path.insert(0, "/opt/trn_rl_repo")

N = 100000
NC = 8
NLOC = N // NC            # 12500
NPAD = 12544              # 98 * 128, per-core padded node count
NCHUNK = 4
# row-groups of local nodes; chunk k's gathered table is the AllGather of
# group k (8 * group rows, < 32768 so int16 indices work).  Groups align
# to the 512-col compute tiles (3072 = 6*512; 3328 = 6*512 + 256).
GROUPS = [3072, 3072, 3072, 3328]
GBASE = [0, 3072, 6144, 9216]
HID = 128
IN0 = 16
N_HIDDEN = 4
OUT_DIM = 3
COLT = 512                # node columns per tile in feature-major compute
N_TRASH = 16              # one trash agg row per index lane
AGG_ROWS = NPAD + 128     # trash rows live at NPAD..NPAD+15

_P = 128
_LANES = 16


def _prep_core(c, src, dst, tw, T):
    """Window-major edge layout: window w (128 dst) gets tw[w] tiles of 128
    edges; gidx[p, t] = m_full row of edge t*128+p (0 for pads), segid[p, t]
    = dst slot within window (-1 for pads)."""
    sel = (dst // NLOC) == c
    s = src[sel].astype(np.int64)
    d = (dst[sel].astype(np.int64)) - c * NLOC
    c_src = s // NLOC
    local = s % NLOC
    gb = np.asarray(GBASE + [NPAD])
    k = np.searchsorted(gb, local, side="right") - 1
    gk = np.asarray(GROUPS)[k]
    srow = 8 * np.asarray(GBASE)[k] + c_src * gk + (local - np.asarray(GBASE)[k])
    w = d // _P
    slot = d % _P
    order = np.argsort(w, kind="stable")
    srow, slot, w = srow[order], slot[order], w[order]
    gidx = np.zeros((T, _P), dtype=np.int32)
    segid = np.full((T, _P), -1.0, dtype=np.float32)
    t0 = 0
    pos = 0
    for wi in range(NPAD // _P):
        n = int(np.searchsorted(w, wi, side="right") - pos)
        cap = tw[wi] * _P
        assert n <= cap, (wi, n, cap)
        flat_g = np.zeros(cap, dtype=np.int32)
        flat_s = np.full(cap, -1.0, dtype=np.float32)
        flat_g[:n] = srow[pos : pos + n]
        flat_s[:n] = slot[pos : pos + n].astype(np.float32)
        gidx[t0 : t0 + tw[wi]] = flat_g.reshape(tw[wi], _P)
        segid[t0 : t0 + tw[wi]] = flat_s.reshape(tw[wi], _P)
        pos += n
        t0 += tw[wi]
    cnt = np.bincount(d, minlength=NPAD).astype(np.float32)
    return np.ascontiguousarray(gidx.T), np.ascontiguousarray(segid.T), cnt


def _window_tiles(src, dst):
    """Per-window tile counts: max over cores (SPMD needs one structure)."""
    nw = NPAD // _P
    tw = np.ones(nw, dtype=np.int64)
    for c in range(NC):
        sel = (dst // NLOC) == c
        d = dst[sel].astype(np.int64) - c * NLOC
        cnts = np.bincount(d // _P, minlength=nw)
        tw = np.maximum(tw, (cnts + _P - 1) // _P)
    return tw


def build_nc(tw):
    """Build the single-program SPMD Bass graph.

    tw: per-window (128 dst nodes) tile counts; sum(tw)*128 padded edges.
    """
    import concourse.bass as bass
    import concourse.bacc as bacc
    import concourse.mybir as mybir
    import concourse.tile as tile
    from concourse.masks import make_identity

    f32 = mybir.dt.float32
    i16 = mybir.dt.int16
    AF = mybir.ActivationFunctionType

    nc = bacc.Bacc(None, num_devices=NC, debug=False)

    T = int(sum(tw))
    nfull = NC * NPAD
    # ---- I/O ----
    h0_in = nc.declare_dram_parameter("h0", [IN0, NPAD], f32, isOutput=False)
    cnt_in = nc.declare_dram_parameter("cntf", [_P, NPAD // _P], f32, isOutput=False)
    gidx_in = nc.declare_dram_parameter("gidx", [_P, T], mybir.dt.int32, isOutput=False)
    seg_in = nc.declare_dram_parameter("segid", [_P, T], f32, isOutput=False)
    iota_in = nc.declare_dram_parameter("iota", [_P, _P], f32, isOutput=False)
    w0_in = nc.declare_dram_parameter("w0", [IN0, HID], f32, isOutput=False)
    wh_in = nc.declare_dram_parameter("whcat", [HID, N_HIDDEN * HID], f32, isOutput=False)
    bg_in = nc.declare_dram_parameter("bgcn", [HID, N_HIDDEN + 1], f32, isOutput=False)
    brep_in = nc.declare_dram_parameter("brep", [_P, HID], f32, isOutput=False)
    wr1_in = nc.declare_dram_parameter("wr1", [HID, HID], f32, isOutput=False)
    br1_in = nc.declare_dram_parameter("br1", [HID, 1], f32, isOutput=False)
    wr2_in = nc.declare_dram_parameter("wr2", [HID, OUT_DIM], f32, isOutput=False)
    br2_in = nc.declare_dram_parameter("br2", [OUT_DIM, 1], f32, isOutput=False)
    emb_out = nc.declare_dram_parameter("emb", [NPAD, HID], f32, isOutput=True)
    pred_out = nc.declare_dram_parameter("pred", [OUT_DIM, NPAD], f32, isOutput=True)

    n_layers = N_HIDDEN + 1
    nchunk = len(GROUPS)
    jmax = COLT // _P  # 4 node sub-tiles per column tile
    col_tiles = []
    c0 = 0
    while c0 < NPAD:
        cw = min(COLT, NPAD - c0)
        # keep tiles within one row-group so each tile feeds one m_loc
        gend = None
        for k in range(nchunk):
            if GBASE[k] <= c0 < GBASE[k] + GROUPS[k]:
                gend = GBASE[k] + GROUPS[k]
        cw = min(cw, gend - c0)
        col_tiles.append((c0, cw))
        c0 += cw

    def group_of(c0):
        for k in range(nchunk):
            if GBASE[k] <= c0 < GBASE[k] + GROUPS[k]:
                return k
        raise AssertionError(c0)

    with tile.TileContext(nc) as tc:
        with (
            tc.tile_pool(name="const", bufs=1) as const,
            tc.tile_pool(name="hbuf", bufs=1) as hbuf,
            tc.tile_pool(name="work", bufs=4) as work,
            tc.tile_pool(name="psA", bufs=2, space="PSUM") as psA,
            tc.tile_pool(name="psB", bufs=1, space="PSUM") as psB,
            tc.tile_pool(name="psC", bufs=2, space="PSUM") as psC,
            tc.tile_pool(name="psD", bufs=2, space="PSUM") as psD,
            tc.tile_pool(name="dram", bufs=2, space="DRAM") as dram,
        ):
            # ---- constants ----
            ident = const.tile([_P, _P], f32, tag="ident")
            make_identity(nc, ident[:])
            w0_sb = const.tile([IN0, HID], f32, tag="w0")
            nc.sync.dma_start(w0_sb[:], w0_in[:])
            wh_sb = const.tile([HID, N_HIDDEN * HID], f32, tag="wh")
            nc.sync.dma_start(wh_sb[:], wh_in[:])
            bg_sb = const.tile([HID, N_HIDDEN + 1], f32, tag="bg")
            nc.sync.dma_start(bg_sb[:], bg_in[:])
            brep_sb = const.tile([_P, HID], f32, tag="brep")
            nc.sync.dma_start(brep_sb[:], brep_in[:])
            wr1_sb = const.tile([HID, HID], f32, tag="wr1")
            nc.sync.dma_start(wr1_sb[:], wr1_in[:])
            br1_sb = const.tile([HID, 1], f32, tag="br1")
            nc.sync.dma_start(br1_sb[:], br1_in[:])
            wr2_sb = const.tile([HID, OUT_DIM], f32, tag="wr2")
            nc.sync.dma_start(wr2_sb[:], wr2_in[:])
            br2_sb = const.tile([OUT_DIM, 1], f32, tag="br2")
            nc.sync.dma_start(br2_sb[:], br2_in[:])
            gidx_sb = const.tile([_P, T], mybir.dt.int32, tag="gidx")
            nc.sync.dma_start(gidx_sb[:], gidx_in[:])
            seg_sb = const.tile([_P, T], f32, tag="segid")
            nc.sync.dma_start(seg_sb[:], seg_in[:])
            iota_sb = const.tile([_P, _P], f32, tag="iota")
            nc.sync.dma_start(iota_sb[:], iota_in[:])

            # deg_inv_sqrt, node-major [128, 98]
            cnt_sb = const.tile([_P, NPAD // _P], f32, tag="cnt")
            nc.sync.dma_start(cnt_sb[:], cnt_in[:])
            sq_sb = const.tile([_P, NPAD // _P], f32, tag="sq")
            nc.scalar.activation(sq_sb[:], cnt_sb[:], AF.Sqrt, bias=1.0)
            dis_sb = const.tile([_P, NPAD // _P], f32, tag="dis")
            nc.vector.reciprocal(dis_sb[:], sq_sb[:])

            # h buffers (one slot, per-layer logical tiles)
            h_tiles = [
                hbuf.tile([HID, NPAD], f32, tag="h", name=f"h{i}")
                for i in range(n_layers + 1)
            ]
            nc.sync.dma_start(h_tiles[0][:IN0, :], h0_in[:])



            for L in range(n_layers):
                kdim = IN0 if L == 0 else HID
                w_ap = w0_sb[:] if L == 0 else wh_sb[:, (L - 1) * HID : L * HID]
                h = h_tiles[L]
                m_loc = [
                    dram.tile([GROUPS[k], HID], f32, tag=f"m_loc{k}",
                              name=f"mloc{k}_{L}")
                    for k in range(nchunk)
                ]
                m_full = dram.tile([nfull, HID], f32, tag="m_full",
                                   name=f"mfull_{L}")
                mn_all = hbuf.tile([_P, NPAD // _P, _P], f32, tag="mn_all",
                                   name=f"mn_all_{L}")

                # ---- A: m = (h @ W) * dis, write node-major m_local + agg init
                for t, (c0, cw) in enumerate(col_tiles):
                    nj = cw // _P
                    pm = psA.tile([_P, COLT], f32, tag="pm")
                    nc.tensor.matmul(
                        pm[:, :cw], lhsT=w_ap, rhs=h[:kdim, c0 : c0 + cw],
                        start=True, stop=True,
                    )
                    mf = work.tile([_P, COLT], f32, tag="mf")
                    nc.vector.tensor_copy(mf[:, :cw], pm[:, :cw])
                    pt = psB.tile([_P, COLT], f32, tag="pt")
                    for j in range(nj):
                        nc.tensor.transpose(
                            pt[:, j * _P : (j + 1) * _P],
                            mf[:, j * _P : (j + 1) * _P],
                            ident[:],
                        )
                    jt0 = c0 // _P
                    for j in range(nj):
                        nc.vector.tensor_scalar_mul(
                            mn_all[:, jt0 + j, :], pt[:, j * _P : (j + 1) * _P],
                            dis_sb[:, jt0 + j : jt0 + j + 1],
                        )
                    kg = group_of(c0)
                    dst_view = m_loc[kg][c0 - GBASE[kg] : c0 - GBASE[kg] + cw, :].rearrange(
                        "(j p) f -> p j f", p=_P
                    )
                    nc.sync.dma_start(dst_view, mn_all[:, jt0 : jt0 + nj, :])

                # ---- C: AllGather per row-group into m_full views
                for k in range(nchunk):
                    base = NC * GBASE[k]
                    nc.gpsimd.collective_compute(
                        "AllGather",
                        mybir.AluOpType.bypass,
                        replica_groups=[list(range(NC))],
                        ins=[m_loc[k].opt()],
                        outs=[m_full[base : base + NC * GROUPS[k], :].opt()],
                    )

                # ---- D+E: per dst-window gather + indicator-matmul
                # accumulate in PSUM, then epilogue straight from PSUM.
                h_next = h_tiles[L + 1]
                last = L == n_layers - 1
                b_ap = bg_sb[:, L : L + 1]
                t0 = 0
                for w in range(NPAD // _P):
                    ntl = int(tw[w])
                    pw = psD.tile([_P, _P], f32, tag="pw", name=f"pw_{L}_{w}")
                    for t in range(ntl):
                        ti = t0 + t
                        rows = work.tile([_P, _P], f32, tag="rows",
                                         name=f"rows_{L}_{ti}")
                        nc.gpsimd.indirect_dma_start(
                            out=rows[:],
                            out_offset=None,
                            in_=m_full[:],
                            in_offset=bass.IndirectOffsetOnAxis(
                                ap=gidx_sb[:, ti : ti + 1], axis=0
                            ),
                        )
                        ind = work.tile([_P, _P], f32, tag="ind",
                                        name=f"ind_{L}_{ti}")
                        nc.vector.tensor_tensor(
                            out=ind[:],
                            in0=seg_sb[:, ti : ti + 1].to_broadcast([_P, _P]),
                            in1=iota_sb[:],
                            op=mybir.AluOpType.is_equal,
                        )
                        nc.tensor.matmul(
                            pw[:], lhsT=ind[:], rhs=rows[:],
                            start=(t == 0), stop=(t == ntl - 1),
                        )
                    t0 += ntl
                    q = work.tile([_P, _P], f32, tag="q", name=f"q_{L}_{w}")
                    nc.vector.tensor_add(q[:], pw[:], mn_all[:, w, :])
                    q2 = work.tile([_P, _P], f32, tag="q2", name=f"q2_{L}_{w}")
                    nc.vector.tensor_scalar_mul(q2[:], q[:], dis_sb[:, w : w + 1])
                    if last:
                        en = work.tile([_P, _P], f32, tag="en", name=f"en_{w}")
                        nc.vector.tensor_add(en[:], q2[:], brep_sb[:])
                        emb_view = emb_out[w * _P : (w + 1) * _P, :]
                        nc.sync.dma_start(emb_view, en[:])
                    pt2 = psC.tile([_P, _P], f32, tag="pt2")
                    nc.tensor.transpose(pt2[:], q2[:], ident[:])
                    nc.scalar.activation(
                        h_next[:, w * _P : (w + 1) * _P], pt2[:], AF.Relu,
                        bias=b_ap,
                    )

            # ---- F: MLP head ----
            h5 = h_tiles[n_layers]
            for t, (c0, cw) in enumerate(col_tiles):
                pm1 = psA.tile([_P, COLT], f32, tag="pm")
                nc.tensor.matmul(
                    pm1[:, :cw], lhsT=wr1_sb[:], rhs=h5[:, c0 : c0 + cw],
                    start=True, stop=True,
                )
                t1 = work.tile([_P, COLT], f32, tag="mf")
                nc.scalar.activation(t1[:, :cw], pm1[:, :cw], AF.Relu, bias=br1_sb[:])
                pm2 = psB.tile([OUT_DIM, COLT], f32, tag="pm2")
                nc.tensor.matmul(
                    pm2[:, :cw], lhsT=wr2_sb[:], rhs=t1[:, :cw],
                    start=True, stop=True,
                )
                pred_t = work.tile([OUT_DIM, COLT], f32, tag="predt")
                nc.scalar.activation(
                    pred_t[:, :cw], pm2[:, :cw], AF.Identity, bias=br2_sb[:]
                )
                nc.sync.dma_start(pred_out[:, c0 : c0 + cw], pred_t[:, :cw])

    nc.compile()
    return nc


def make_inputs(x, x_mask, edge_index, W0, b0, Wh, bh, Wr1, br1, Wr2, br2, tw):
    src = np.asarray(edge_index[0], dtype=np.int64)
    dst = np.asarray(edge_index[1], dtype=np.int64)
    T = int(sum(tw))
    hcat = np.concatenate(
        [np.asarray(x)[:, :8], np.asarray(x_mask)[:, :8]], axis=1
    ).astype(np.float32)

    whcat = np.concatenate([np.asarray(Wh[i]) for i in range(N_HIDDEN)], axis=1)
    bgcn = np.stack(
        [np.asarray(b0)] + [np.asarray(bh[i]) for i in range(N_HIDDEN)], axis=1
    ).astype(np.float32)
    brep = np.tile(np.asarray(bh[N_HIDDEN - 1])[None, :], (_P, 1)).astype(np.float32)
    iota = np.tile(np.arange(_P, dtype=np.float32)[None, :], (_P, 1))

    in_maps = []
    for c in range(NC):
        gidx, segid, cnt = _prep_core(c, src, dst, tw, T)
        h0 = np.zeros((IN0, NPAD), dtype=np.float32)
        h0[:, :NLOC] = hcat[c * NLOC : (c + 1) * NLOC].T
        cntf = cnt.reshape(NPAD // _P, _P).T.copy()  # [128, 98] node-major
        in_maps.append(
            dict(
                h0=np.ascontiguousarray(h0),
                cntf=np.ascontiguousarray(cntf),
                gidx=gidx,
                segid=segid,
                iota=np.ascontiguousarray(iota),
                w0=np.ascontiguousarray(np.asarray(W0, dtype=np.float32)),
                whcat=np.ascontiguousarray(whcat.astype(np.float32)),
                bgcn=np.ascontiguousarray(bgcn),
                brep=np.ascontiguousarray(brep),
                wr1=np.ascontiguousarray(np.asarray(Wr1, dtype=np.float32)),
                br1=np.ascontiguousarray(
                    np.asarray(br1, dtype=np.float32).reshape(HID, 1)
                ),
                wr2=np.ascontiguousarray(np.asarray(Wr2, dtype=np.float32)),
                br2=np.ascontiguousarray(
                    np.asarray(br2, dtype=np.float32).reshape(OUT_DIM, 1)
                ),
            )
        )
    return in_maps


def run(inputs, trace=False):
    from concourse import bass_utils

    src = np.asarray(inputs["edge_index"][0], dtype=np.int64)
    dst = np.asarray(inputs["edge_index"][1], dtype=np.int64)
    tw = _window_tiles(src, dst)

    nc = build_nc(tw)
    in_maps = make_inputs(
        inputs["x"], inputs["x_mask"], inputs["edge_index"],
        inputs["W0"], inputs["b0"], inputs["Wh"], inputs["bh"],
        inputs["Wr1"], inputs["br1"], inputs["Wr2"], inputs["br2"],
        tw,
    )
    res = bass_utils.run_bass_kernel_spmd(
        nc, in_maps, core_ids=list(range(NC)), trace=trace
    )
    emb = np.empty((N, HID), dtype=np.float32)
    pred = np.empty((N, OUT_DIM), dtype=np.float32)
    for c in range(NC):
        emb[c * NLOC : (c + 1) * NLOC] = res.results[c]["emb"][:NLOC]
        pred[c * NLOC : (c + 1) * NLOC] = res.results[c]["pred"][:, :NLOC].T
    return (emb, pred), res


def kernel(**inputs):
    out, _ = run(inputs, trace=False)
    return out
